# revision 24
# baseline (speedup 1.0000x reference)
"""Bass/Tile kernel for nn_MultiHeadAttention (B=2, S=2048, D=1024, H=16) on 8 trn2 cores.

Sharding: core c -> (b = c//4, head-group hg = c%4). Each core computes 4 heads'
q/k/v projections, relu-attention, and a partial FC (256 of 1024 contraction rows).
Host pre-casts to bf16, pre-arranges x / weight slices, and sums the 4
partials per batch + bias.

v7 design notes (on top of v6):
  - measured: an unblocked scores- or av-quadrant pair streams in 216ns total,
    so the attention loop is DRAIN-bound, not PE-bound. The drain floor is two
    relu engines (V 1223ns / S 1113ns per 1024-wide tile).
  - per m-iteration ONE 1024-wide relu (whole head-pair) on V or S picked by a
    greedy per-engine load balancer; saves the 168ns/instruction overhead of
    512-wide pairs.
  - PSUM: one 3-slot pool of [128, 2*512] units (6 banks) shared by scores
    pairs, kq c-pair units, v quad units and fc units; 2 banks for av po.
    3-deep rotation gives ~1.8us slot cycle >> 1.28us relu latency: no
    recycle stalls.
  - kq/v/fc copybacks are 1024-wide (one per unit).
  - dma issues split across scalar (critical x/wk stream; its queue is ready
    ~1.2us before sync's) and gpsimd (wq/wv/x3/wfc); y stores rotate over
    sync/scalar/gpsimd queues.
  - tail: split po copyback + split fc copies (V||S) + half-stores on two
    queues.
"""
import numpy as np
import ml_dtypes

import concourse.bass as bass
import concourse.mybir as mybir
import concourse.tile as tile

F32 = mybir.dt.float32
BF16 = mybir.dt.bfloat16
ts, ds = bass.ts, bass.ds

S = 2048
D = 1024
DL = 256      # per-core q/k/v dim (4 heads x 64)
P = 128
KD = D // P   # 8 k-chunks for projections
SQ = 512      # q-block (matmul N)
NQB = S // SQ # 4
NM = S // P   # 16 kpos chunks
DLC = DL // P # 2


def split_excess_waits(nc, max_embed: int = 1):
    """walrus core_v3 codegen accepts at most one sync-wait per instruction;
    move extra waits onto standalone event-sem instructions inserted before."""
    n_split = 0
    counter = 0
    for f in nc.m.functions:
        for blk in f.blocks:
            insts = blk.instructions
            if not any(
                ins.sync_info is not None and len(ins.sync_info.on_wait) > max_embed
                for ins in insts
            ):
                continue
            newl = []
            for ins in insts:
                si = ins.sync_info
                if si is not None and len(si.on_wait) > max_embed:
                    waits = list(si.on_wait)
                    extra, keep = waits[:-max_embed], waits[-max_embed:]
                    for w in extra:
                        counter += 1
                        es = mybir.InstEventSemaphore(name=f"waitsplit_{counter}")
                        es.engine = ins.engine
                        es.sync_info = mybir.SyncInfo(on_wait=[w], on_update=[])
                        newl.append(es)
                        n_split += 1
                    si.on_wait = keep
                newl.append(ins)
            blk.instructions = newl
    return n_split


def build_nc(with_mask: bool):
    nc = bass.Bass()
    # pre-arranged on host: x[p, nb, k, sq] = x.T[128k+p, 512nb+sq] (one
    # contiguous 1MB block per q-block); w[p, c, f] = w.T[128c+p, f]
    xT = nc.dram_tensor("xT", [P, NQB, KD, SQ], BF16, kind="ExternalInput")
    wq = nc.dram_tensor("wq", [P, DLC, KD, P], BF16, kind="ExternalInput")
    wk = nc.dram_tensor("wk", [P, DLC, KD, P], BF16, kind="ExternalInput")
    wv = nc.dram_tensor("wv", [P, KD, DL], BF16, kind="ExternalInput")
    wfc = nc.dram_tensor("wfc", [P, DLC, D], BF16, kind="ExternalInput")
    maskT = nc.dram_tensor("maskT", [S, S], F32, kind="ExternalInput") if with_mask else None
    # bf16 output halves the store traffic; partials are summed in fp64 on host
    y = nc.dram_tensor("y", [S, D], BF16, kind="ExternalOutput")

    with tile.TileContext(nc) as tc:
        _Emitter(tc, xT, wq, wk, wv, wfc, maskT, y).run()
    split_excess_waits(nc)
    return nc


class _Emitter:
    def __init__(self, tc, xT, wq, wk, wv, wfc, maskT, y):
        self.tc = tc
        self.nc = tc.nc
        self.xT, self.wq, self.wk, self.wv, self.wfc = xT, wq, wk, wv, wfc
        self.maskT, self.y = maskT, y
        # greedy V/S load balance (ns of queued work per engine)
        self.eng_ns = {"v": 0.0, "s": 0.0}
        self.store_qs = None

    # -- engine load balancing ---------------------------------------------
    def pick_engine(self, v_cost, s_cost):
        """pick the engine that finishes this op sooner; return (name, cost)."""
        if self.eng_ns["v"] + v_cost <= self.eng_ns["s"] + s_cost:
            self.eng_ns["v"] += v_cost
            return "v"
        self.eng_ns["s"] += s_cost
        return "s"

    def relu1024(self, out_ap, u):
        nc = self.nc
        eng = self.pick_engine(1223.0, 1113.0)
        if eng == "v":
            nc.vector.tensor_scalar_max(out_ap, u, 0.0)
        else:
            nc.scalar.activation(out_ap, u, mybir.ActivationFunctionType.Relu)

    def copy_wide(self, out_ap, in_ap, width):
        nc = self.nc
        cost = 170.0 + width  # ~1ns/col + fixed
        eng = self.pick_engine(cost, cost)
        if eng == "v":
            nc.vector.tensor_copy(out_ap, in_ap)
        else:
            nc.scalar.copy(out_ap, in_ap)

    def next_store_q(self):
        q = self.store_qs[0]
        self.store_qs = self.store_qs[1:] + [q]
        return q

    # -- emission pieces ----------------------------------------------------
    def kq_unit(self, wsb, dstT, nb):
        """projection unit: both c-halves of dstT[:, :, nb*SQ:] + one 1024 copy"""
        nc = self.nc
        u = self.ps_u.tile([P, 2, SQ], F32, tag="u", name=f"pj_{dstT.name}_{nb}")
        for c in range(DLC):
            for k in range(KD):
                nc.tensor.matmul(
                    u[:, c, :], wsb[:, c, k, :], self.xb[:, nb, k, :],
                    start=(k == 0), stop=(k == KD - 1),
                )
        self.copy_wide(dstT[:, :, ds(nb * SQ, SQ)], u[:], 1024)

    def kq_half(self, wsb, dstT, c, nb):
        """c-granular projection action (for injected qT work): 8 mms + 512 copy.
        Uses the dedicated injection bank so the scores rotation is untouched."""
        nc = self.nc
        u = self.ps_inj.tile([P, SQ], F32, tag="inj", name=f"pjh_{dstT.name}_{c}_{nb}")
        for k in range(KD):
            nc.tensor.matmul(
                u[:], wsb[:, c, k, :], self.xb[:, nb, k, :],
                start=(k == 0), stop=(k == KD - 1),
            )
        self.copy_wide(dstT[:, c, ds(nb * SQ, SQ)], u[:], 512)

    def v_action(self, sc, head=False):
        """one v s-chunk: 8 mms (N=256) + 256-wide copy. Head-phase chunks
        use the (then idle) scores rotation so they pipeline; injected ones
        use the dedicated injection bank."""
        nc = self.nc
        if head:
            u = self.ps_u.tile([P, 2, SQ], F32, tag="u", name=f"v_{sc}")[:, 0, :]
        else:
            u = self.ps_inj.tile([P, SQ], F32, tag="inj", name=f"v_{sc}")[:]
        for k in range(KD):
            nc.tensor.matmul(
                u[0:P, 0:DL], self.xb[:, sc // 4, k, ds((sc % 4) * P, P)],
                self.wv_sb[:, k, :],
                start=(k == 0), stop=(k == KD - 1),
            )
        self.copy_wide(self.vN[:, sc, :], u[0:P, 0:DL], 256)

    def scores_pair(self, qb, hp, m, attn_t, mtile):
        """both heads' score matmuls run concurrently in disjoint PE
        row-quadrants into one 2-bank unit; ONE 1024-wide relu drains it."""
        nc = self.nc
        u = self.ps_u.tile([P, 2, SQ], F32, tag="u", name=f"sc_{qb}_{hp}_{m}")
        for h in range(2):
            nc.tensor.matmul(
                u[:, h, :],
                self.kT[ds(64 * h, 64), hp, ts(m, P)],
                self.qT[ds(64 * h, 64), hp, ds(qb * SQ, SQ)],
                start=True, stop=True,
            )
        if mtile is not None:
            mb = mtile[:, m, :].unsqueeze(1).broadcast_to([P, 2, SQ])
            nc.vector.tensor_tensor(u[:], u[:], mb, mybir.AluOpType.add)
            self.eng_ns["v"] += 1223.0
        self.relu1024(attn_t[:, m, :, :], u[:])

    def av(self, qb, hp, m, attn_t, po):
        nc = self.nc
        for h in range(2):
            nc.tensor.matmul(
                po[ds(64 * h, 64), :],
                self.vN[:, m, ds(128 * hp + 64 * h, 64)],
                attn_t[:, m, h, :],
                start=(m == 0), stop=(m == NM - 1),
            )

    def fc_double(self, sc):
        """injected fc action for one s-chunk: two eb halves run sequentially
        through the injection bank; the store fires once both are staged."""
        nc = self.nc
        yt = self.ystage.tile([P, 2, SQ], BF16, tag="yt", name=f"yt_{sc}")
        for eb in range(2):
            u = self.ps_inj.tile([P, SQ], F32, tag="inj", name=f"fc_{sc}_{eb}")
            for c in range(DLC):
                nc.tensor.matmul(
                    u[:], self.outT[:, c, ts(sc, P)],
                    self.wfc_sb[:, c, ds(eb * SQ, SQ)],
                    start=(c == 0), stop=(c == DLC - 1),
                )
            self.copy_wide(yt[:, eb, :], u[:], 512)
        self.next_store_q().dma_start(
            self.y[ts(sc, P), :].rearrange("p (e q) -> p e q", e=2), yt[:])

    def fc_tail(self, sc):
        """tail fc: scores pool is free, use a full-width unit with split
        copies and two half stores so the ring drains immediately"""
        nc = self.nc
        u = self.ps_u.tile([P, 2, SQ], F32, tag="u", name=f"fct_{sc}")
        for eb in range(2):
            for c in range(DLC):
                nc.tensor.matmul(
                    u[:, eb, :], self.outT[:, c, ts(sc, P)],
                    self.wfc_sb[:, c, ds(eb * SQ, SQ)],
                    start=(c == 0), stop=(c == DLC - 1),
                )
        yt = self.ystage.tile([P, 2, SQ], BF16, tag="yt", name=f"yt_{sc}")
        nc.vector.tensor_copy(yt[:, 0, :], u[:, 0, :])
        nc.scalar.copy(yt[:, 1, :], u[:, 1, :])
        self.next_store_q().dma_start(self.y[ts(sc, P), 0:SQ], yt[:, 0, :])
        self.next_store_q().dma_start(self.y[ts(sc, P), SQ:2 * SQ], yt[:, 1, :])

    def inject(self):
        if self.pe_pending:
            self.pe_pending.pop(0)()

    def load_mask(self, qb):
        if self.maskT is None:
            return None
        nc = self.nc
        mtile = self.mstg.tile([P, NM, SQ], F32, tag="mask", name=f"mask_{qb}")
        for m in range(NM):
            nc.sync.dma_start(
                mtile[:, m, :],
                self.maskT[:, :].rearrange("(m p) q -> p m q", p=P)[:, m, ds(qb * SQ, SQ)],
            )
        return mtile

    # -- main ---------------------------------------------------------------
    def run(self):
        from contextlib import ExitStack

        tc, nc = self.tc, self.nc
        self.store_qs = [nc.sync, nc.scalar, nc.gpsimd]
        stack = ExitStack()
        sb = stack.enter_context(tc.tile_pool(name="sb", bufs=1))
        # PSUM budget (8 banks): scores 3x2 (dedicated rotation so the WAW
        # chain stays 3 iterations deep), po 1 (single-buffered: the software
        # pipeline copies it out before reuse), injected units 1
        self.ps_u = stack.enter_context(tc.tile_pool(name="ps_u", bufs=3, space="PSUM"))
        self.ps_av = stack.enter_context(tc.tile_pool(name="ps_av", bufs=1, space="PSUM"))
        self.ps_inj = stack.enter_context(tc.tile_pool(name="ps_inj", bufs=1, space="PSUM"))
        self.attn_pool = stack.enter_context(tc.tile_pool(name="attn", bufs=2))
        self.mstg = stack.enter_context(tc.tile_pool(name="mstg", bufs=2))
        self.ystage = stack.enter_context(tc.tile_pool(name="ystage", bufs=3))

        self.xb = sb.tile([P, NQB, KD, SQ], BF16, name="xb")
        self.wq_sb = sb.tile([P, DLC, KD, P], BF16, name="wq_sb")
        self.wk_sb = sb.tile([P, DLC, KD, P], BF16, name="wk_sb")
        self.wv_sb = sb.tile([P, KD, DL], BF16, name="wv_sb")
        self.wfc_sb = sb.tile([P, DLC, D], BF16, name="wfc_sb")
        self.qT = sb.tile([P, DLC, S], BF16, name="qT")
        self.kT = sb.tile([P, DLC, S], BF16, name="kT")
        self.vN = sb.tile([P, NM, DL], BF16, name="vN")
        self.outT = sb.tile([P, DLC, S], BF16, name="outT")
        self.pe_pending = []

        # loads: sync hw queue (empirically ~2x the bandwidth of the scalar /
        # gpsimd queues) carries the critical wk/x stream in consumption
        # order; the scalar queue carries the secondary weights + x3.
        sy, sca = nc.sync, nc.scalar
        sy.dma_start(self.wk_sb[:, 0, ds(0, 4), :], self.wk[:, 0, ds(0, 4), :])
        sy.dma_start(self.xb[:, 0, ds(0, 4), :], self.xT[:, 0, ds(0, 4), :])
        sy.dma_start(self.wk_sb[:, 0, ds(4, 4), :], self.wk[:, 0, ds(4, 4), :])
        sy.dma_start(self.xb[:, 0, ds(4, 4), :], self.xT[:, 0, ds(4, 4), :])
        sy.dma_start(self.wk_sb[:, 1, :, :], self.wk[:, 1, :, :])
        for kh in range(2):
            sy.dma_start(self.xb[:, 1, ds(kh * 4, 4), :], self.xT[:, 1, ds(kh * 4, 4), :])
        for kh in range(2):
            sy.dma_start(self.xb[:, 2, ds(kh * 4, 4), :], self.xT[:, 2, ds(kh * 4, 4), :])
        sca.dma_start(self.wq_sb[:], self.wq[:, :, :, :])
        sca.dma_start(self.wv_sb[:], self.wv[:, :, :])
        for kh in range(2):
            sca.dma_start(self.xb[:, 3, ds(kh * 4, 4), :], self.xT[:, 3, ds(kh * 4, 4), :])
        sca.dma_start(self.wfc_sb[:], self.wfc[:, :, :])

        # PE p-state warmup: dummy matmuls on a memset scratch tile run
        # during the otherwise-idle DMA wait so the ramp to 2.4GHz happens
        # before the first real projection
        scratch = sb.tile([P, SQ], BF16, name="scratch")
        nc.vector.memset(scratch[:], 0.0)
        wp = self.ps_inj.tile([P, SQ], F32, tag="inj", name="warmup")
        for _ in range(12):
            nc.tensor.matmul(wp[:], scratch[:, 0:P], scratch[:], start=True, stop=True)

        # serial head in arrival order: only what the first loop strictly
        # needs up front (kT, qT0, v0-9); everything else is injected into
        # the attention loops' PE slack (they are drain-bound).
        self.kq_unit(self.wk_sb, self.kT, 0)
        self.kq_unit(self.wk_sb, self.kT, 1)
        self.kq_unit(self.wk_sb, self.kT, 2)
        self.kq_unit(self.wq_sb, self.qT, 0)
        for sc in range(6):
            self.v_action(sc, head=True)
        self.kq_unit(self.wk_sb, self.kT, 3)
        for sc in range(6, 13):
            self.v_action(sc, head=True)
        self.pe_pending = [lambda sc=sc: self.v_action(sc) for sc in range(13, 16)]

        # attention loops are software-pipelined ACROSS (qb, hp): each loop's
        # last two av steps and its po copyback slide into the next loop's
        # first two iterations, so the PE stream never drains at boundaries
        def po_copyback(pqb, php, ppo):
            nc.vector.tensor_copy(
                self.outT[:, php, ds(pqb * SQ, SQ // 2)], ppo[:, 0:SQ // 2])
            nc.scalar.copy(
                self.outT[:, php, ds(pqb * SQ + SQ // 2, SQ // 2)],
                ppo[:, SQ // 2:SQ])
            self.eng_ns["v"] += 424.0
            self.eng_ns["s"] += 424.0

        prev = None
        for qb in range(NQB):
            mt = self.load_mask(qb)
            for hp in range(DLC):
                at = self.attn_pool.tile(
                    [P, NM, 2, SQ], BF16, tag="attn", name=f"attn_{qb}_{hp}")
                po = self.ps_av.tile([P, SQ], F32, tag="av", name=f"av_{qb}_{hp}")
                # batch-4 geometry: 4 scores pairs then 4 av pairs — each
                # row-quadrant <-> col-quadrant tile switch costs ~100ns of PE
                # pipeline drain, so batching cuts the switch count 4x
                for bi in range(4):
                    for j in range(4):
                        self.scores_pair(qb, hp, 4 * bi + j, at, mt)
                    if bi == 0:
                        if prev is not None:
                            pqb, php, pat, ppo = prev
                            for j in range(4):
                                self.av(pqb, php, NM - 4 + j, pat, ppo)
                            po_copyback(pqb, php, ppo)
                    else:
                        for j in range(4):
                            self.av(qb, hp, 4 * (bi - 1) + j, at, po)
                    # injections use their own psum bank (never the scores
                    # rotation); one per batch boundary
                    if bi < 3:
                        self.inject()
                prev = (qb, hp, at, po)
                if hp == 0 and qb < NQB - 1:
                    qn = qb + 1
                    self.pe_pending[:0] = [
                        (lambda c=c, qn=qn: self.kq_half(self.wq_sb, self.qT, c, qn))
                        for c in range(DLC)
                    ]
            if qb < NQB - 1:
                self.pe_pending += [
                    (lambda sc=sc: self.fc_double(sc))
                    for sc in range(qb * 4, qb * 4 + 4)
                ]
        # drain the final loop's av tail, then its fc
        qb3, hp3, at3, po3 = prev
        for j in range(4):
            self.av(qb3, hp3, NM - 4 + j, at3, po3)
        po_copyback(qb3, hp3, po3)
        self.pe_pending += [
            (lambda sc=sc: self.fc_tail(sc))
            for sc in range(12, 16)
        ]
        while self.pe_pending:
            self.inject()

        stack.close()


# ---- host wrapper ---------------------------------------------------------

N_HEAD = 16
_nc_cache = {}


def get_nc(with_mask: bool):
    if with_mask not in _nc_cache:
        _nc_cache[with_mask] = build_nc(with_mask)
    return _nc_cache[with_mask]


def make_in_maps(x, mask, Wq, Wk, Wv, Wfc, with_mask):
    scale = np.float32(1.0 / np.sqrt(D // N_HEAD))
    bf = ml_dtypes.bfloat16
    in_maps = []
    for c in range(8):
        b, hg = divmod(c, 4)
        gs = slice(DL * hg, DL * hg + DL)
        def prearrange(wT, cdim):  # [cdim*128, F] -> [128, cdim, F]
            F = wT.shape[1]
            return np.ascontiguousarray(
                wT.reshape(cdim, P, F).transpose(1, 0, 2)
            ).astype(bf)

        def prearrange_c(wT):  # [KD*128, DLC*128] -> [128, DLC, KD, 128]
            return np.ascontiguousarray(
                wT.reshape(KD, P, DLC, P).transpose(1, 2, 0, 3)
            ).astype(bf)

        xt = x[b].T.reshape(KD, P, NQB, SQ).transpose(1, 2, 0, 3)
        m = {
            "xT": np.ascontiguousarray(xt).astype(bf),
            "wq": prearrange_c((Wq[gs, :] * scale).T),
            "wk": prearrange_c(Wk[gs, :].T),
            "wv": prearrange(Wv[gs, :].T, KD),
            "wfc": prearrange(Wfc[:, gs].T, DLC),
        }
        if with_mask:
            m["maskT"] = np.ascontiguousarray(
                np.broadcast_to(mask, (1, 1, S, S))[0, 0].T.astype(np.float32)
            )
        in_maps.append(m)
    return in_maps


def kernel(x, mask, Wq, Wk, Wv, Wfc, bfc):
    """Full-input entry: shards across 8 trn2 cores, returns the full output."""
    from concourse.bass_utils import run_bass_kernel_spmd

    x = np.asarray(x, dtype=np.float32)
    mask = np.asarray(mask, dtype=np.float32)
    Wq = np.asarray(Wq, dtype=np.float32)
    Wk = np.asarray(Wk, dtype=np.float32)
    Wv = np.asarray(Wv, dtype=np.float32)
    Wfc = np.asarray(Wfc, dtype=np.float32)
    bfc = np.asarray(bfc, dtype=np.float32)

    B = x.shape[0]
    with_mask = bool(np.any(mask))
    nc = get_nc(with_mask)
    in_maps = make_in_maps(x, mask, Wq, Wk, Wv, Wfc, with_mask)

    res = run_bass_kernel_spmd(nc, in_maps, core_ids=list(range(8)))
    parts = np.stack([np.asarray(r["y"], dtype=np.float64) for r in res.results])
    out = parts.reshape(B, 4, S, D).sum(axis=1)
    out += bfc.astype(np.float64)
    return out.astype(np.float32)


# revision 26
# speedup vs baseline: 1.0657x; 1.0657x over previous
"""Bass/Tile kernel for nn_MultiHeadAttention (B=2, S=2048, D=1024, H=16) on 8 trn2 cores.

Sharding: core c -> (b = c//4, head-group hg = c%4). Each core computes 4 heads'
q/k/v projections, relu-attention, and a partial FC (256 of 1024 contraction rows).
Host pre-casts to bf16, pre-arranges x / weight slices, and sums the 4
partials per batch + bias.

v6 design notes:
  - scores: the two heads of a head-pair run CONCURRENTLY in disjoint 64-row
    PE quadrants (tile_position from base partitions), writing one 2-bank
    PSUM tile; ONE 1024-wide relu drains the pair. Vector gets 7 and Scalar 9
    of the 16 relus per m-loop (scalar is faster per element).
  - av: two heads run concurrently in disjoint 64-col output quadrants.
  - attention m-loops are relu/PE balanced (~9.4us each); all remaining PE
    work (v-projections, fc, next q-projection) is injected into the loops.
  - fc computed as (sc)-pairs: 4 matmuls -> 2-bank PSUM -> one 1024-wide
    gpsimd cast-copy -> one 256KB bf16 store with 2KB lines.
  - x arrives as 4 contiguous 1MB nb-block DMAs (8KB/partition lines) on the
    sync hardware queue behind wk; wq+wv ride the scalar hardware queue.
  - y is bf16; host sums partials in fp64.
"""
import numpy as np
import ml_dtypes

import concourse.bass as bass
import concourse.mybir as mybir
import concourse.tile as tile

F32 = mybir.dt.float32
BF16 = mybir.dt.bfloat16
ts, ds = bass.ts, bass.ds

S = 2048
D = 1024
DL = 256      # per-core q/k/v dim (4 heads x 64)
P = 128
KD = D // P   # 8 k-chunks for projections
SQ = 512      # q-block (matmul N)
NQB = S // SQ # 4
NM = S // P   # 16 kpos chunks
DLC = DL // P # 2


def split_excess_waits(nc, max_embed: int = 1):
    """walrus core_v3 codegen accepts at most one sync-wait per instruction;
    move extra waits onto standalone event-sem instructions inserted before."""
    n_split = 0
    counter = 0
    for f in nc.m.functions:
        for blk in f.blocks:
            insts = blk.instructions
            if not any(
                ins.sync_info is not None and len(ins.sync_info.on_wait) > max_embed
                for ins in insts
            ):
                continue
            newl = []
            for ins in insts:
                si = ins.sync_info
                if si is not None and len(si.on_wait) > max_embed:
                    waits = list(si.on_wait)
                    extra, keep = waits[:-max_embed], waits[-max_embed:]
                    for w in extra:
                        counter += 1
                        es = mybir.InstEventSemaphore(name=f"waitsplit_{counter}")
                        es.engine = ins.engine
                        es.sync_info = mybir.SyncInfo(on_wait=[w], on_update=[])
                        newl.append(es)
                        n_split += 1
                    si.on_wait = keep
                newl.append(ins)
            blk.instructions = newl
    return n_split


def build_nc(with_mask: bool):
    nc = bass.Bass()
    # pre-arranged on host: x[p, nb, k, sq] = x.T[128k+p, 512nb+sq] (one
    # contiguous 1MB block per q-block); w[p, c, f] = w.T[128c+p, f]
    xT = nc.dram_tensor("xT", [P, NQB, KD, SQ], BF16, kind="ExternalInput")
    wq = nc.dram_tensor("wq", [P, DLC, KD, P], BF16, kind="ExternalInput")
    wk = nc.dram_tensor("wk", [P, DLC, KD, P], BF16, kind="ExternalInput")
    wv = nc.dram_tensor("wv", [P, KD, DL], BF16, kind="ExternalInput")
    wfc = nc.dram_tensor("wfc", [P, DLC, D], BF16, kind="ExternalInput")
    maskT = nc.dram_tensor("maskT", [S, S], F32, kind="ExternalInput") if with_mask else None
    # bf16 output halves the store traffic; partials are summed in fp64 on host
    y = nc.dram_tensor("y", [S, D], BF16, kind="ExternalOutput")

    with tile.TileContext(nc) as tc:
        _Emitter(tc, xT, wq, wk, wv, wfc, maskT, y).run()
    split_excess_waits(nc)
    return nc


class _Emitter:
    def __init__(self, tc, xT, wq, wk, wv, wfc, maskT, y):
        self.tc = tc
        self.nc = tc.nc
        self.xT, self.wq, self.wk, self.wv, self.wfc = xT, wq, wk, wv, wfc
        self.maskT, self.y = maskT, y
        self.cp = 0

    # -- engine helpers -----------------------------------------------------
    def copyback(self, out_ap, in_ap):
        if self.cp % 2 == 0:
            self.nc.vector.tensor_copy(out_ap, in_ap)
        else:
            self.nc.scalar.copy(out_ap, in_ap)
        self.cp += 1

    # -- emission pieces ----------------------------------------------------
    def kq_group(self, wsb, dstT, c, nb):
        """one projection psum group: dstT[:, c, nb*SQ:...] via 8 k-chunk matmuls"""
        nc = self.nc
        pt = self.ps_fc.tile([P, SQ], F32, tag="fc", name=f"pj_{dstT.name}_{c}_{nb}")
        for k in range(KD):
            nc.tensor.matmul(
                pt[:], wsb[:, c, k, :], self.xb[:, nb, k, :],
                start=(k == 0), stop=(k == KD - 1),
            )
        self.copyback(dstT[:, c, ds(nb * SQ, SQ)], pt[:])

    def v_group(self, sc):
        nc = self.nc
        pt = self.ps_fc.tile([P, DL], F32, tag="fc", name=f"v_{sc}")
        for k in range(KD):
            nc.tensor.matmul(
                pt[:], self.xb[:, sc // 4, k, ds((sc % 4) * P, P)], self.wv_sb[:, k, :],
                start=(k == 0), stop=(k == KD - 1),
            )
        self.copyback(self.vN[:, sc, :], pt[:])

    def scores_pair(self, qb, hp, m, attn_t, mtile):
        """both heads' score matmuls run concurrently in disjoint PE
        row-quadrants; separate 1-bank tiles + 512-wide relus keep the
        drain latency low enough for the 4-slot psum rotation"""
        nc = self.nc
        pts = []
        for h in range(2):
            pt = self.ps_sc.tile([P, SQ], F32, tag="sc", name=f"sc_{qb}_{hp}_{m}_{h}")
            nc.tensor.matmul(
                pt[:],
                self.kT[ds(64 * h, 64), hp, ts(m, P)],
                self.qT[ds(64 * h, 64), hp, ds(qb * SQ, SQ)],
                start=True, stop=True,
            )
            pts.append(pt)
        for h in range(2):
            pt = pts[h]
            if mtile is not None:
                nc.vector.tensor_tensor(
                    pt[:], pt[:], mtile[:, m, :], mybir.AluOpType.add
                )
            if h == 0:
                nc.vector.tensor_scalar_max(attn_t[:, m, h, :], pt[:], 0.0)
            else:
                nc.scalar.activation(
                    attn_t[:, m, h, :], pt[:], mybir.ActivationFunctionType.Relu)

    def av(self, qb, hp, m, attn_t, po):
        nc = self.nc
        for h in range(2):
            nc.tensor.matmul(
                po[ds(64 * h, 64), :],
                self.vN[:, m, ds(128 * hp + 64 * h, 64)],
                attn_t[:, m, h, :],
                start=(m == 0), stop=(m == NM - 1),
            )

    def fc_group(self, sc, eb, tail=False):
        """fc for one (s-chunk, column-half): 2 matmuls -> 1-bank psum ->
        cast into the shared [P,2,SQ] stage; eb==1 fires one 256KB store"""
        nc = self.nc
        pool = self.ps_sc if (tail and (sc + eb) % 2 == 0) else self.ps_fc
        pt = pool.tile([P, SQ], F32, tag="sc" if pool is self.ps_sc else "fc",
                       name=f"fc_{sc}_{eb}")
        for c in range(DLC):
            nc.tensor.matmul(
                pt[:], self.outT[:, c, ts(sc, P)],
                self.wfc_sb[:, c, ds(eb * SQ, SQ)],
                start=(c == 0), stop=(c == DLC - 1),
            )
        if eb == 0:
            self.yt_cur = self.ystage.tile([P, 2, SQ], BF16, tag="yt", name=f"yt_{sc}")
        yt = self.yt_cur
        if tail:
            # engines are free at the tail; split copies and store each half
            # immediately so the ring drains during the remaining matmuls
            nc.vector.tensor_copy(yt[:, eb, 0:SQ // 2], pt[:, 0:SQ // 2])
            nc.scalar.copy(yt[:, eb, SQ // 2:SQ], pt[:, SQ // 2:SQ])
            nc.sync.dma_start(self.y[ts(sc, P), ds(eb * SQ, SQ)], yt[:, eb, :])
        else:
            self.copyback(yt[:, eb, :], pt[:])
            if eb == 1:
                nc.sync.dma_start(
                    self.y[ts(sc, P), :].rearrange("p (e q) -> p e q", e=2),
                    yt[:, :, :])

    def inject(self):
        if self.pe_pending:
            self.pe_pending.pop(0)()

    def load_mask(self, qb):
        if self.maskT is None:
            return None
        nc = self.nc
        mtile = self.mstg.tile([P, NM, SQ], F32, tag="mask", name=f"mask_{qb}")
        for m in range(NM):
            nc.sync.dma_start(
                mtile[:, m, :],
                self.maskT[:, :].rearrange("(m p) q -> p m q", p=P)[:, m, ds(qb * SQ, SQ)],
            )
        return mtile

    # -- main ---------------------------------------------------------------
    def run(self):
        from contextlib import ExitStack

        tc, nc = self.tc, self.nc
        stack = ExitStack()
        sb = stack.enter_context(tc.tile_pool(name="sb", bufs=1))
        # PSUM budget (8 banks): sc 4 (score double-buffer), fc 2
        # (projections + injected work), av 2 (po double buffer)
        self.ps_sc = stack.enter_context(tc.tile_pool(name="ps_sc", bufs=4, space="PSUM"))
        self.ps_fc = stack.enter_context(tc.tile_pool(name="ps_fc", bufs=2, space="PSUM"))
        self.ps_av = stack.enter_context(tc.tile_pool(name="ps_av", bufs=2, space="PSUM"))
        self.attn_pool = stack.enter_context(tc.tile_pool(name="attn", bufs=2))
        self.mstg = stack.enter_context(tc.tile_pool(name="mstg", bufs=2))
        self.ystage = stack.enter_context(tc.tile_pool(name="ystage", bufs=3))

        self.xb = sb.tile([P, NQB, KD, SQ], BF16, name="xb")
        self.wq_sb = sb.tile([P, DLC, KD, P], BF16, name="wq_sb")
        self.wk_sb = sb.tile([P, DLC, KD, P], BF16, name="wk_sb")
        self.wv_sb = sb.tile([P, KD, DL], BF16, name="wv_sb")
        self.wfc_sb = sb.tile([P, DLC, D], BF16, name="wfc_sb")
        self.qT = sb.tile([P, DLC, S], BF16, name="qT")
        self.kT = sb.tile([P, DLC, S], BF16, name="kT")
        self.vN = sb.tile([P, NM, DL], BF16, name="vN")
        self.outT = sb.tile([P, DLC, S], BF16, name="outT")
        self.pe_pending = []

        # loads: ONE fifo queue in exact consumption order (ring arbitration
        # would otherwise let later transfers steal bandwidth from the x
        # stream the head is waiting on). The head emission below matches
        # this order so the PE's in-order stream never waits on a transfer
        # queued behind data it already consumed.
        def ld(dst, src):
            nc.sync.dma_start(dst, src)

        ld(self.wk_sb[:, 0, :, :], self.wk[:, 0, :, :])
        ld(self.xb[:, 0, ds(0, 4), :], self.xT[:, 0, ds(0, 4), :])
        ld(self.wk_sb[:, 1, :, :], self.wk[:, 1, :, :])
        ld(self.xb[:, 0, ds(4, 4), :], self.xT[:, 0, ds(4, 4), :])
        for kh in range(2):
            ld(self.xb[:, 1, ds(kh * 4, 4), :], self.xT[:, 1, ds(kh * 4, 4), :])
        ld(self.wq_sb[:], self.wq[:, :, :, :])
        for kh in range(2):
            ld(self.xb[:, 2, ds(kh * 4, 4), :], self.xT[:, 2, ds(kh * 4, 4), :])
        ld(self.wv_sb[:], self.wv[:, :, :])
        for kh in range(2):
            ld(self.xb[:, 3, ds(kh * 4, 4), :], self.xT[:, 3, ds(kh * 4, 4), :])
        ld(self.wfc_sb[:], self.wfc[:, :, :])

        # PE p-state warmup: dummy matmuls on a memset scratch tile run
        # during the otherwise-idle DMA wait so the ramp to 2.4GHz happens
        # before the first real projection
        scratch = sb.tile([P, SQ], BF16, name="scratch")
        nc.vector.memset(scratch[:], 0.0)
        wp = self.ps_av.tile([P, SQ], F32, tag="av", name="warmup")
        for _ in range(12):
            nc.tensor.matmul(wp[:], scratch[:, 0:P], scratch[:], start=True, stop=True)

        # serial head in arrival order: kT nb0-2, q-projection for qb0,
        # v chunks 0-11, kT nb3, v chunks 12-15
        for nb in range(3):
            for c in range(DLC):
                self.kq_group(self.wk_sb, self.kT, c, nb)
        for c in range(DLC):
            self.kq_group(self.wq_sb, self.qT, c, 0)
        for sc in range(12):
            self.v_group(sc)
        for c in range(DLC):
            self.kq_group(self.wk_sb, self.kT, c, 3)
        for sc in range(12, NM):
            self.v_group(sc)

        # attention loops are software-pipelined ACROSS (qb, hp): each loop's
        # last two av steps and its po copyback slide into the next loop's
        # first two iterations, so the PE stream never drains at boundaries
        def po_copyback(pqb, php, ppo):
            nc.vector.tensor_copy(
                self.outT[:, php, ds(pqb * SQ, SQ // 2)], ppo[:, 0:SQ // 2])
            nc.scalar.copy(
                self.outT[:, php, ds(pqb * SQ + SQ // 2, SQ // 2)],
                ppo[:, SQ // 2:SQ])

        prev = None
        for qb in range(NQB):
            mt = self.load_mask(qb)
            for hp in range(DLC):
                at = self.attn_pool.tile(
                    [P, NM, 2, SQ], BF16, tag="attn", name=f"attn_{qb}_{hp}")
                po = self.ps_av.tile([P, SQ], F32, tag="av", name=f"av_{qb}_{hp}")
                # batch-2 geometry: two scores pairs, then two av pairs.
                # A row-quadrant <-> col-quadrant weight-tile switch costs
                # ~100ns of PE pipeline drain; batching halves the switches
                # (measured 656 -> 557 ns per iteration pure-PE), which drops
                # the PE floor below the relu-drain floor and gives injected
                # work slack to amortize into.
                for mb in range(NM // 2):
                    m0, m1 = 2 * mb, 2 * mb + 1
                    self.scores_pair(qb, hp, m0, at, mt)
                    self.scores_pair(qb, hp, m1, at, mt)
                    if mb == 0:
                        if prev is not None:
                            pqb, php, pat, ppo = prev
                            self.av(pqb, php, NM - 2, pat, ppo)
                            self.av(pqb, php, NM - 1, pat, ppo)
                            po_copyback(pqb, php, ppo)
                    else:
                        self.av(qb, hp, m0 - 2, at, po)
                        self.av(qb, hp, m1 - 2, at, po)
                        self.inject()
                prev = (qb, hp, at, po)
                if hp == 0 and qb < NQB - 1:
                    qn = qb + 1
                    self.pe_pending[:0] = [
                        (lambda c=c, qn=qn: self.kq_group(self.wq_sb, self.qT, c, qn))
                        for c in range(DLC)
                    ]
            if qb < NQB - 1:
                self.pe_pending += [
                    (lambda sc=sc, eb=eb: self.fc_group(sc, eb, tail=False))
                    for sc in range(qb * 4, qb * 4 + 4) for eb in range(2)
                ]
        # drain the final loop's av tail, then its fc
        qb3, hp3, at3, po3 = prev
        for m in (NM, NM + 1):
            self.av(qb3, hp3, m - 2, at3, po3)
        po_copyback(qb3, hp3, po3)
        self.pe_pending += [
            (lambda sc=sc, eb=eb: self.fc_group(sc, eb, tail=True))
            for sc in range(12, 16) for eb in range(2)
        ]
        while self.pe_pending:
            self.inject()

        stack.close()


# ---- host wrapper ---------------------------------------------------------

N_HEAD = 16
_nc_cache = {}


def get_nc(with_mask: bool):
    if with_mask not in _nc_cache:
        _nc_cache[with_mask] = build_nc(with_mask)
    return _nc_cache[with_mask]


def make_in_maps(x, mask, Wq, Wk, Wv, Wfc, with_mask):
    scale = np.float32(1.0 / np.sqrt(D // N_HEAD))
    bf = ml_dtypes.bfloat16
    in_maps = []
    for c in range(8):
        b, hg = divmod(c, 4)
        gs = slice(DL * hg, DL * hg + DL)
        def prearrange(wT, cdim):  # [cdim*128, F] -> [128, cdim, F]
            F = wT.shape[1]
            return np.ascontiguousarray(
                wT.reshape(cdim, P, F).transpose(1, 0, 2)
            ).astype(bf)

        def prearrange_c(wT):  # [KD*128, DLC*128] -> [128, DLC, KD, 128]
            return np.ascontiguousarray(
                wT.reshape(KD, P, DLC, P).transpose(1, 2, 0, 3)
            ).astype(bf)

        xt = x[b].T.reshape(KD, P, NQB, SQ).transpose(1, 2, 0, 3)
        m = {
            "xT": np.ascontiguousarray(xt).astype(bf),
            "wq": prearrange_c((Wq[gs, :] * scale).T),
            "wk": prearrange_c(Wk[gs, :].T),
            "wv": prearrange(Wv[gs, :].T, KD),
            "wfc": prearrange(Wfc[:, gs].T, DLC),
        }
        if with_mask:
            m["maskT"] = np.ascontiguousarray(
                np.broadcast_to(mask, (1, 1, S, S))[0, 0].T.astype(np.float32)
            )
        in_maps.append(m)
    return in_maps


def kernel(x, mask, Wq, Wk, Wv, Wfc, bfc):
    """Full-input entry: shards across 8 trn2 cores, returns the full output."""
    from concourse.bass_utils import run_bass_kernel_spmd

    x = np.asarray(x, dtype=np.float32)
    mask = np.asarray(mask, dtype=np.float32)
    Wq = np.asarray(Wq, dtype=np.float32)
    Wk = np.asarray(Wk, dtype=np.float32)
    Wv = np.asarray(Wv, dtype=np.float32)
    Wfc = np.asarray(Wfc, dtype=np.float32)
    bfc = np.asarray(bfc, dtype=np.float32)

    B = x.shape[0]
    with_mask = bool(np.any(mask))
    nc = get_nc(with_mask)
    in_maps = make_in_maps(x, mask, Wq, Wk, Wv, Wfc, with_mask)

    res = run_bass_kernel_spmd(nc, in_maps, core_ids=list(range(8)))
    parts = np.stack([np.asarray(r["y"], dtype=np.float64) for r in res.results])
    out = parts.reshape(B, 4, S, D).sum(axis=1)
    out += bfc.astype(np.float64)
    return out.astype(np.float32)



# revision 27
# speedup vs baseline: 1.2680x; 1.1899x over previous
"""Bass/Tile kernel for nn_MultiHeadAttention (B=2, S=2048, D=1024, H=16) on 8 trn2 cores.

Sharding: core c -> (b = c//4, head-group hg = c%4). Each core computes 4 heads'
q/k/v projections, relu-attention, and a partial FC (256 of 1024 contraction rows).
Host pre-casts to bf16, pre-arranges x / weight slices, and sums the 4
partials per batch + bias.

v6 design notes:
  - scores: the two heads of a head-pair run CONCURRENTLY in disjoint 64-row
    PE quadrants (tile_position from base partitions), writing one 2-bank
    PSUM tile; ONE 1024-wide relu drains the pair. Vector gets 7 and Scalar 9
    of the 16 relus per m-loop (scalar is faster per element).
  - av: two heads run concurrently in disjoint 64-col output quadrants.
  - attention m-loops are relu/PE balanced (~9.4us each); all remaining PE
    work (v-projections, fc, next q-projection) is injected into the loops.
  - fc computed as (sc)-pairs: 4 matmuls -> 2-bank PSUM -> one 1024-wide
    gpsimd cast-copy -> one 256KB bf16 store with 2KB lines.
  - x arrives as 4 contiguous 1MB nb-block DMAs (8KB/partition lines) on the
    sync hardware queue behind wk; wq+wv ride the scalar hardware queue.
  - y is bf16; host sums partials in fp64.
"""
import numpy as np
import ml_dtypes

import concourse.bass as bass
import concourse.mybir as mybir
import concourse.tile as tile

F32 = mybir.dt.float32
BF16 = mybir.dt.bfloat16
ts, ds = bass.ts, bass.ds

S = 2048
D = 1024
DL = 256      # per-core q/k/v dim (4 heads x 64)
P = 128
KD = D // P   # 8 k-chunks for projections
SQ = 512      # q-block (matmul N)
NQB = S // SQ # 4
NM = S // P   # 16 kpos chunks
DLC = DL // P # 2


def split_excess_waits(nc, max_embed: int = 1):
    """walrus core_v3 codegen accepts at most one sync-wait per instruction;
    move extra waits onto standalone event-sem instructions inserted before."""
    n_split = 0
    counter = 0
    for f in nc.m.functions:
        for blk in f.blocks:
            insts = blk.instructions
            if not any(
                ins.sync_info is not None and len(ins.sync_info.on_wait) > max_embed
                for ins in insts
            ):
                continue
            newl = []
            for ins in insts:
                si = ins.sync_info
                if si is not None and len(si.on_wait) > max_embed:
                    waits = list(si.on_wait)
                    extra, keep = waits[:-max_embed], waits[-max_embed:]
                    for w in extra:
                        counter += 1
                        es = mybir.InstEventSemaphore(name=f"waitsplit_{counter}")
                        es.engine = ins.engine
                        es.sync_info = mybir.SyncInfo(on_wait=[w], on_update=[])
                        newl.append(es)
                        n_split += 1
                    si.on_wait = keep
                newl.append(ins)
            blk.instructions = newl
    return n_split


def build_nc(with_mask: bool):
    nc = bass.Bass()
    # pre-arranged on host: x[p, nb, k, sq] = x.T[128k+p, 512nb+sq] (one
    # contiguous 1MB block per q-block); w[p, c, f] = w.T[128c+p, f]
    xT = nc.dram_tensor("xT", [P, NQB, KD, SQ], BF16, kind="ExternalInput")
    wq = nc.dram_tensor("wq", [P, DLC, KD, P], BF16, kind="ExternalInput")
    wk = nc.dram_tensor("wk", [P, DLC, KD, P], BF16, kind="ExternalInput")
    wv = nc.dram_tensor("wv", [P, KD, DL], BF16, kind="ExternalInput")
    wfc = nc.dram_tensor("wfc", [P, DLC, D], BF16, kind="ExternalInput")
    maskT = nc.dram_tensor("maskT", [S, S], F32, kind="ExternalInput") if with_mask else None
    # bf16 output halves the store traffic; partials are summed in fp64 on host
    y = nc.dram_tensor("y", [S, D], BF16, kind="ExternalOutput")

    with tile.TileContext(nc) as tc:
        _Emitter(tc, xT, wq, wk, wv, wfc, maskT, y).run()
    split_excess_waits(nc)
    return nc


class _Emitter:
    def __init__(self, tc, xT, wq, wk, wv, wfc, maskT, y):
        self.tc = tc
        self.nc = tc.nc
        self.xT, self.wq, self.wk, self.wv, self.wfc = xT, wq, wk, wv, wfc
        self.maskT, self.y = maskT, y
        self.cp = 0

    # -- engine helpers -----------------------------------------------------
    def copyback(self, out_ap, in_ap):
        if self.cp % 2 == 0:
            self.nc.vector.tensor_copy(out_ap, in_ap)
        else:
            self.nc.scalar.copy(out_ap, in_ap)
        self.cp += 1

    # -- emission pieces ----------------------------------------------------
    def kq_group(self, wsb, dstT, c, nb):
        """one projection psum group: dstT[:, c, nb*SQ:...] via 8 k-chunk matmuls"""
        nc = self.nc
        pt = self.ps_fc.tile([P, SQ], F32, tag="fc", name=f"pj_{dstT.name}_{c}_{nb}")
        for k in range(KD):
            nc.tensor.matmul(
                pt[:], wsb[:, c, k, :], self.xb[:, nb, k, :],
                start=(k == 0), stop=(k == KD - 1),
            )
        self.copyback(dstT[:, c, ds(nb * SQ, SQ)], pt[:])

    def v_group(self, sc):
        nc = self.nc
        pt = self.ps_fc.tile([P, DL], F32, tag="fc", name=f"v_{sc}")
        for k in range(KD):
            nc.tensor.matmul(
                pt[:], self.xb[:, sc // 4, k, ds((sc % 4) * P, P)], self.wv_sb[:, k, :],
                start=(k == 0), stop=(k == KD - 1),
            )
        self.copyback(self.vN[:, sc, :], pt[:])

    def scores_pair(self, qb, hp, m, attn_t, mtile):
        """both heads' score matmuls run concurrently in disjoint PE
        row-quadrants; separate 1-bank tiles + 512-wide relus keep the
        drain latency low enough for the 4-slot psum rotation"""
        nc = self.nc
        pts = []
        for h in range(2):
            pt = self.ps_sc.tile([P, SQ], F32, tag="sc", name=f"sc_{qb}_{hp}_{m}_{h}")
            nc.tensor.matmul(
                pt[:],
                self.kT[ds(64 * h, 64), hp, ts(m, P)],
                self.qT[ds(64 * h, 64), hp, ds(qb * SQ, SQ)],
                start=True, stop=True,
            )
            pts.append(pt)
        for h in range(2):
            pt = pts[h]
            if mtile is not None:
                nc.vector.tensor_tensor(
                    pt[:], pt[:], mtile[:, m, :], mybir.AluOpType.add
                )
            if h == 0:
                nc.vector.tensor_scalar_max(attn_t[:, m, h, :], pt[:], 0.0)
            else:
                nc.scalar.activation(
                    attn_t[:, m, h, :], pt[:], mybir.ActivationFunctionType.Relu)

    def av(self, qb, hp, m, attn_t, po):
        nc = self.nc
        for h in range(2):
            nc.tensor.matmul(
                po[ds(64 * h, 64), :],
                self.vN[:, m, ds(128 * hp + 64 * h, 64)],
                attn_t[:, m, h, :],
                start=(m == 0), stop=(m == NM - 1),
            )

    def fc_group(self, sc, eb, tail=False):
        """fc for one (s-chunk, column-half): 2 matmuls -> 1-bank psum ->
        cast into the shared [P,2,SQ] stage; eb==1 fires one 256KB store"""
        nc = self.nc
        pool = self.ps_sc if (tail and (sc + eb) % 2 == 0) else self.ps_fc
        pt = pool.tile([P, SQ], F32, tag="sc" if pool is self.ps_sc else "fc",
                       name=f"fc_{sc}_{eb}")
        for c in range(DLC):
            nc.tensor.matmul(
                pt[:], self.outT[:, c, ts(sc, P)],
                self.wfc_sb[:, c, ds(eb * SQ, SQ)],
                start=(c == 0), stop=(c == DLC - 1),
            )
        if eb == 0:
            self.yt_cur = self.ystage.tile([P, 2, SQ], BF16, tag="yt", name=f"yt_{sc}")
        yt = self.yt_cur
        if tail:
            # engines are free at the tail; split copies and store each half
            # immediately so the ring drains during the remaining matmuls
            nc.vector.tensor_copy(yt[:, eb, 0:SQ // 2], pt[:, 0:SQ // 2])
            nc.scalar.copy(yt[:, eb, SQ // 2:SQ], pt[:, SQ // 2:SQ])
            nc.sync.dma_start(self.y[ts(sc, P), ds(eb * SQ, SQ)], yt[:, eb, :])
        else:
            self.copyback(yt[:, eb, :], pt[:])
            if eb == 1:
                nc.sync.dma_start(
                    self.y[ts(sc, P), :].rearrange("p (e q) -> p e q", e=2),
                    yt[:, :, :])

    def inject(self):
        if self.pe_pending:
            self.pe_pending.pop(0)()

    def load_mask(self, qb):
        if self.maskT is None:
            return None
        nc = self.nc
        mtile = self.mstg.tile([P, NM, SQ], F32, tag="mask", name=f"mask_{qb}")
        for m in range(NM):
            nc.sync.dma_start(
                mtile[:, m, :],
                self.maskT[:, :].rearrange("(m p) q -> p m q", p=P)[:, m, ds(qb * SQ, SQ)],
            )
        return mtile

    # -- main ---------------------------------------------------------------
    def run(self):
        from contextlib import ExitStack

        tc, nc = self.tc, self.nc
        stack = ExitStack()
        sb = stack.enter_context(tc.tile_pool(name="sb", bufs=1))
        # PSUM budget (8 banks): sc 4 (score double-buffer), fc 2
        # (projections + injected work), av 2 (po double buffer)
        self.ps_sc = stack.enter_context(tc.tile_pool(name="ps_sc", bufs=4, space="PSUM"))
        self.ps_fc = stack.enter_context(tc.tile_pool(name="ps_fc", bufs=2, space="PSUM"))
        self.ps_av = stack.enter_context(tc.tile_pool(name="ps_av", bufs=2, space="PSUM"))
        self.attn_pool = stack.enter_context(tc.tile_pool(name="attn", bufs=2))
        self.mstg = stack.enter_context(tc.tile_pool(name="mstg", bufs=2))
        self.ystage = stack.enter_context(tc.tile_pool(name="ystage", bufs=3))

        self.xb = sb.tile([P, NQB, KD, SQ], BF16, name="xb")
        self.wq_sb = sb.tile([P, DLC, KD, P], BF16, name="wq_sb")
        self.wk_sb = sb.tile([P, DLC, KD, P], BF16, name="wk_sb")
        self.wv_sb = sb.tile([P, KD, DL], BF16, name="wv_sb")
        self.wfc_sb = sb.tile([P, DLC, D], BF16, name="wfc_sb")
        self.qT = sb.tile([P, DLC, S], BF16, name="qT")
        self.kT = sb.tile([P, DLC, S], BF16, name="kT")
        self.vN = sb.tile([P, NM, DL], BF16, name="vN")
        self.outT = sb.tile([P, DLC, S], BF16, name="outT")
        self.pe_pending = []

        # loads: ONE fifo queue in exact consumption order (ring arbitration
        # would otherwise let later transfers steal bandwidth from the x
        # stream the head is waiting on). The head emission below matches
        # this order so the PE's in-order stream never waits on a transfer
        # queued behind data it already consumed.
        def ld(dst, src):
            nc.sync.dma_start(dst, src)

        ld(self.wk_sb[:, 0, :, :], self.wk[:, 0, :, :])
        ld(self.xb[:, 0, ds(0, 4), :], self.xT[:, 0, ds(0, 4), :])
        ld(self.wk_sb[:, 1, :, :], self.wk[:, 1, :, :])
        ld(self.xb[:, 0, ds(4, 4), :], self.xT[:, 0, ds(4, 4), :])
        for kh in range(2):
            ld(self.xb[:, 1, ds(kh * 4, 4), :], self.xT[:, 1, ds(kh * 4, 4), :])
        ld(self.wq_sb[:], self.wq[:, :, :, :])
        for kh in range(2):
            ld(self.xb[:, 2, ds(kh * 4, 4), :], self.xT[:, 2, ds(kh * 4, 4), :])
        ld(self.wv_sb[:], self.wv[:, :, :])
        for kh in range(2):
            ld(self.xb[:, 3, ds(kh * 4, 4), :], self.xT[:, 3, ds(kh * 4, 4), :])
        ld(self.wfc_sb[:], self.wfc[:, :, :])

        # PE p-state warmup: dummy matmuls on a memset scratch tile run
        # during the otherwise-idle DMA wait so the ramp to 2.4GHz happens
        # before the first real projection
        scratch = sb.tile([P, SQ], BF16, name="scratch")
        nc.vector.memset(scratch[:], 0.0)
        wp = self.ps_av.tile([P, SQ], F32, tag="av", name="warmup")
        for _ in range(12):
            nc.tensor.matmul(wp[:], scratch[:, 0:P], scratch[:], start=True, stop=True)

        # serial head in arrival order: kT nb0-2, q-projection for qb0,
        # v chunks 0-11, kT nb3, v chunks 12-15
        for nb in range(3):
            for c in range(DLC):
                self.kq_group(self.wk_sb, self.kT, c, nb)
        self.kq_group(self.wq_sb, self.qT, 0, 0)
        for sc in range(11):
            self.v_group(sc)
        for c in range(DLC):
            self.kq_group(self.wk_sb, self.kT, c, 3)
        # qT0-c1 and v11-15 ride loop 0's otherwise-empty injection slots
        # (batch-2 leaves ~126ns/iteration of PE slack under the relu floor);
        # deadlines: v(m) is needed by av(m) two iterations later at the
        # earliest, qT0-c1 by loop (0,1)'s first scores pair.
        self.pe_pending = [
            lambda: self.kq_group(self.wq_sb, self.qT, 1, 0)
        ] + [lambda sc=sc: self.v_group(sc) for sc in range(11, NM)]

        # attention loops are software-pipelined ACROSS (qb, hp): each loop's
        # last two av steps and its po copyback slide into the next loop's
        # first two iterations, so the PE stream never drains at boundaries
        def po_copyback(pqb, php, ppo):
            nc.vector.tensor_copy(
                self.outT[:, php, ds(pqb * SQ, SQ // 2)], ppo[:, 0:SQ // 2])
            nc.scalar.copy(
                self.outT[:, php, ds(pqb * SQ + SQ // 2, SQ // 2)],
                ppo[:, SQ // 2:SQ])

        prev = None
        for qb in range(NQB):
            mt = self.load_mask(qb)
            for hp in range(DLC):
                at = self.attn_pool.tile(
                    [P, NM, 2, SQ], BF16, tag="attn", name=f"attn_{qb}_{hp}")
                po = self.ps_av.tile([P, SQ], F32, tag="av", name=f"av_{qb}_{hp}")
                # batch-2 geometry: two scores pairs, then two av pairs.
                # A row-quadrant <-> col-quadrant weight-tile switch costs
                # ~100ns of PE pipeline drain; batching halves the switches
                # (measured 656 -> 557 ns per iteration pure-PE), which drops
                # the PE floor below the relu-drain floor and gives injected
                # work slack to amortize into.
                for mb in range(NM // 2):
                    m0, m1 = 2 * mb, 2 * mb + 1
                    self.scores_pair(qb, hp, m0, at, mt)
                    self.scores_pair(qb, hp, m1, at, mt)
                    if mb == 0:
                        if prev is not None:
                            pqb, php, pat, ppo = prev
                            self.av(pqb, php, NM - 2, pat, ppo)
                            self.av(pqb, php, NM - 1, pat, ppo)
                            po_copyback(pqb, php, ppo)
                    else:
                        self.av(qb, hp, m0 - 2, at, po)
                        self.av(qb, hp, m1 - 2, at, po)
                        self.inject()
                prev = (qb, hp, at, po)
                if hp == 0 and qb < NQB - 1:
                    qn = qb + 1
                    self.pe_pending[:0] = [
                        (lambda c=c, qn=qn: self.kq_group(self.wq_sb, self.qT, c, qn))
                        for c in range(DLC)
                    ]
            if qb < NQB - 1:
                self.pe_pending += [
                    (lambda sc=sc, eb=eb: self.fc_group(sc, eb, tail=False))
                    for sc in range(qb * 4, qb * 4 + 4) for eb in range(2)
                ]
        # drain the final loop's av tail, then its fc
        qb3, hp3, at3, po3 = prev
        for m in (NM, NM + 1):
            self.av(qb3, hp3, m - 2, at3, po3)
        po_copyback(qb3, hp3, po3)
        self.pe_pending += [
            (lambda sc=sc, eb=eb: self.fc_group(sc, eb, tail=True))
            for sc in range(12, 16) for eb in range(2)
        ]
        while self.pe_pending:
            self.inject()

        stack.close()


# ---- host wrapper ---------------------------------------------------------

N_HEAD = 16
_nc_cache = {}


def get_nc(with_mask: bool):
    if with_mask not in _nc_cache:
        _nc_cache[with_mask] = build_nc(with_mask)
    return _nc_cache[with_mask]


def make_in_maps(x, mask, Wq, Wk, Wv, Wfc, with_mask):
    scale = np.float32(1.0 / np.sqrt(D // N_HEAD))
    bf = ml_dtypes.bfloat16
    in_maps = []
    for c in range(8):
        b, hg = divmod(c, 4)
        gs = slice(DL * hg, DL * hg + DL)
        def prearrange(wT, cdim):  # [cdim*128, F] -> [128, cdim, F]
            F = wT.shape[1]
            return np.ascontiguousarray(
                wT.reshape(cdim, P, F).transpose(1, 0, 2)
            ).astype(bf)

        def prearrange_c(wT):  # [KD*128, DLC*128] -> [128, DLC, KD, 128]
            return np.ascontiguousarray(
                wT.reshape(KD, P, DLC, P).transpose(1, 2, 0, 3)
            ).astype(bf)

        xt = x[b].T.reshape(KD, P, NQB, SQ).transpose(1, 2, 0, 3)
        m = {
            "xT": np.ascontiguousarray(xt).astype(bf),
            "wq": prearrange_c((Wq[gs, :] * scale).T),
            "wk": prearrange_c(Wk[gs, :].T),
            "wv": prearrange(Wv[gs, :].T, KD),
            "wfc": prearrange(Wfc[:, gs].T, DLC),
        }
        if with_mask:
            m["maskT"] = np.ascontiguousarray(
                np.broadcast_to(mask, (1, 1, S, S))[0, 0].T.astype(np.float32)
            )
        in_maps.append(m)
    return in_maps


def kernel(x, mask, Wq, Wk, Wv, Wfc, bfc):
    """Full-input entry: shards across 8 trn2 cores, returns the full output."""
    from concourse.bass_utils import run_bass_kernel_spmd

    x = np.asarray(x, dtype=np.float32)
    mask = np.asarray(mask, dtype=np.float32)
    Wq = np.asarray(Wq, dtype=np.float32)
    Wk = np.asarray(Wk, dtype=np.float32)
    Wv = np.asarray(Wv, dtype=np.float32)
    Wfc = np.asarray(Wfc, dtype=np.float32)
    bfc = np.asarray(bfc, dtype=np.float32)

    B = x.shape[0]
    with_mask = bool(np.any(mask))
    nc = get_nc(with_mask)
    in_maps = make_in_maps(x, mask, Wq, Wk, Wv, Wfc, with_mask)

    res = run_bass_kernel_spmd(nc, in_maps, core_ids=list(range(8)))
    parts = np.stack([np.asarray(r["y"], dtype=np.float64) for r in res.results])
    out = parts.reshape(B, 4, S, D).sum(axis=1)
    out += bfc.astype(np.float64)
    return out.astype(np.float32)



# revision 30
# speedup vs baseline: 1.2767x; 1.0069x over previous
"""Bass/Tile kernel for nn_MultiHeadAttention (B=2, S=2048, D=1024, H=16) on 8 trn2 cores.

Sharding: core c -> (b = c//4, head-group hg = c%4). Each core computes 4 heads'
q/k/v projections, relu-attention, and a partial FC (256 of 1024 contraction rows).
Host pre-casts to bf16, pre-arranges x / weight slices, and sums the 4
partials per batch + bias.

v6 design notes:
  - scores: the two heads of a head-pair run CONCURRENTLY in disjoint 64-row
    PE quadrants (tile_position from base partitions), writing one 2-bank
    PSUM tile; ONE 1024-wide relu drains the pair. Vector gets 7 and Scalar 9
    of the 16 relus per m-loop (scalar is faster per element).
  - av: two heads run concurrently in disjoint 64-col output quadrants.
  - attention m-loops are relu/PE balanced (~9.4us each); all remaining PE
    work (v-projections, fc, next q-projection) is injected into the loops.
  - fc computed as (sc)-pairs: 4 matmuls -> 2-bank PSUM -> one 1024-wide
    gpsimd cast-copy -> one 256KB bf16 store with 2KB lines.
  - x arrives as 4 contiguous 1MB nb-block DMAs (8KB/partition lines) on the
    sync hardware queue behind wk; wq+wv ride the scalar hardware queue.
  - y is bf16; host sums partials in fp64.
"""
import numpy as np
import ml_dtypes

import concourse.bass as bass
import concourse.mybir as mybir
import concourse.tile as tile

F32 = mybir.dt.float32
BF16 = mybir.dt.bfloat16
ts, ds = bass.ts, bass.ds

S = 2048
D = 1024
DL = 256      # per-core q/k/v dim (4 heads x 64)
P = 128
KD = D // P   # 8 k-chunks for projections
SQ = 512      # q-block (matmul N)
NQB = S // SQ # 4
NM = S // P   # 16 kpos chunks
DLC = DL // P # 2


def split_excess_waits(nc, max_embed: int = 1):
    """walrus core_v3 codegen accepts at most one sync-wait per instruction;
    move extra waits onto standalone event-sem instructions inserted before."""
    n_split = 0
    counter = 0
    for f in nc.m.functions:
        for blk in f.blocks:
            insts = blk.instructions
            if not any(
                ins.sync_info is not None and len(ins.sync_info.on_wait) > max_embed
                for ins in insts
            ):
                continue
            newl = []
            for ins in insts:
                si = ins.sync_info
                if si is not None and len(si.on_wait) > max_embed:
                    waits = list(si.on_wait)
                    extra, keep = waits[:-max_embed], waits[-max_embed:]
                    for w in extra:
                        counter += 1
                        es = mybir.InstEventSemaphore(name=f"waitsplit_{counter}")
                        es.engine = ins.engine
                        es.sync_info = mybir.SyncInfo(on_wait=[w], on_update=[])
                        newl.append(es)
                        n_split += 1
                    si.on_wait = keep
                newl.append(ins)
            blk.instructions = newl
    return n_split


def build_nc(with_mask: bool):
    nc = bass.Bass()
    # pre-arranged on host: x[p, nb, k, sq] = x.T[128k+p, 512nb+sq] (one
    # contiguous 1MB block per q-block); w[p, c, f] = w.T[128c+p, f]
    xT = nc.dram_tensor("xT", [P, NQB, KD, SQ], BF16, kind="ExternalInput")
    wq = nc.dram_tensor("wq", [P, DLC, KD, P], BF16, kind="ExternalInput")
    wk = nc.dram_tensor("wk", [P, DLC, KD, P], BF16, kind="ExternalInput")
    wv = nc.dram_tensor("wv", [P, KD, DL], BF16, kind="ExternalInput")
    wfc = nc.dram_tensor("wfc", [P, DLC, D], BF16, kind="ExternalInput")
    maskT = nc.dram_tensor("maskT", [S, S], F32, kind="ExternalInput") if with_mask else None
    # bf16 output halves the store traffic; partials are summed in fp64 on host
    y = nc.dram_tensor("y", [S, D], BF16, kind="ExternalOutput")

    with tile.TileContext(nc) as tc:
        _Emitter(tc, xT, wq, wk, wv, wfc, maskT, y).run()
    split_excess_waits(nc)
    return nc


class _Emitter:
    def __init__(self, tc, xT, wq, wk, wv, wfc, maskT, y):
        self.tc = tc
        self.nc = tc.nc
        self.xT, self.wq, self.wk, self.wv, self.wfc = xT, wq, wk, wv, wfc
        self.maskT, self.y = maskT, y
        self.cp = 0

    # -- engine helpers -----------------------------------------------------
    def copyback(self, out_ap, in_ap):
        if self.cp % 2 == 0:
            self.nc.vector.tensor_copy(out_ap, in_ap)
        else:
            self.nc.scalar.copy(out_ap, in_ap)
        self.cp += 1

    def next_store_q(self):
        q = self.store_qs[0]
        self.store_qs = self.store_qs[1:] + [q]
        return q

    # -- emission pieces ----------------------------------------------------
    def kq_group(self, wsb, dstT, c, nb):
        """one projection psum group: dstT[:, c, nb*SQ:...] via 8 k-chunk matmuls"""
        nc = self.nc
        pt = self.ps_fc.tile([P, SQ], F32, tag="fc", name=f"pj_{dstT.name}_{c}_{nb}")
        for k in range(KD):
            nc.tensor.matmul(
                pt[:], wsb[:, c, k, :], self.xb[:, nb, k, :],
                start=(k == 0), stop=(k == KD - 1),
            )
        self.copyback(dstT[:, c, ds(nb * SQ, SQ)], pt[:])

    def v_group(self, sc):
        nc = self.nc
        pt = self.ps_fc.tile([P, DL], F32, tag="fc", name=f"v_{sc}")
        for k in range(KD):
            nc.tensor.matmul(
                pt[:], self.xb[:, sc // 4, k, ds((sc % 4) * P, P)], self.wv_sb[:, k, :],
                start=(k == 0), stop=(k == KD - 1),
            )
        self.copyback(self.vN[:, sc, :], pt[:])

    def scores_pair(self, qb, hp, m, attn_t, mtile):
        """both heads' score matmuls run concurrently in disjoint PE
        row-quadrants; separate 1-bank tiles + 512-wide relus keep the
        drain latency low enough for the 4-slot psum rotation"""
        nc = self.nc
        pts = []
        for h in range(2):
            pt = self.ps_sc.tile([P, SQ], F32, tag="sc", name=f"sc_{qb}_{hp}_{m}_{h}")
            nc.tensor.matmul(
                pt[:],
                self.kT[ds(64 * h, 64), hp, ts(m, P)],
                self.qT[ds(64 * h, 64), hp, ds(qb * SQ, SQ)],
                start=True, stop=True,
            )
            pts.append(pt)
        for h in range(2):
            pt = pts[h]
            if mtile is not None:
                nc.vector.tensor_tensor(
                    pt[:], pt[:], mtile[:, m, :], mybir.AluOpType.add
                )
            if h == 0:
                nc.vector.tensor_scalar_max(attn_t[:, m, h, :], pt[:], 0.0)
            else:
                nc.scalar.activation(
                    attn_t[:, m, h, :], pt[:], mybir.ActivationFunctionType.Relu)

    def av(self, qb, hp, m, attn_t, po):
        nc = self.nc
        for h in range(2):
            nc.tensor.matmul(
                po[ds(64 * h, 64), :],
                self.vN[:, m, ds(128 * hp + 64 * h, 64)],
                attn_t[:, m, h, :],
                start=(m == 0), stop=(m == NM - 1),
            )

    def fc_group(self, sc, eb, tail=False):
        """fc for one (s-chunk, column-half): 2 matmuls -> 1-bank psum ->
        cast into the shared [P,2,SQ] stage; eb==1 fires one 256KB store"""
        nc = self.nc
        pool = self.ps_sc if (tail and (sc + eb) % 2 == 0) else self.ps_fc
        pt = pool.tile([P, SQ], F32, tag="sc" if pool is self.ps_sc else "fc",
                       name=f"fc_{sc}_{eb}")
        for c in range(DLC):
            nc.tensor.matmul(
                pt[:], self.outT[:, c, ts(sc, P)],
                self.wfc_sb[:, c, ds(eb * SQ, SQ)],
                start=(c == 0), stop=(c == DLC - 1),
            )
        if eb == 0:
            self.yt_cur = self.ystage.tile([P, 2, SQ], BF16, tag="yt", name=f"yt_{sc}")
        yt = self.yt_cur
        if tail:
            # engines are free at the tail; split copies and store each half
            # immediately so the ring drains during the remaining matmuls.
            # Stores rotate over the three DMA-capable queues so the ~600ns
            # issue costs and the transfers themselves run in parallel.
            nc.vector.tensor_copy(yt[:, eb, 0:SQ // 2], pt[:, 0:SQ // 2])
            nc.scalar.copy(yt[:, eb, SQ // 2:SQ], pt[:, SQ // 2:SQ])
            self.next_store_q().dma_start(
                self.y[ts(sc, P), ds(eb * SQ, SQ)], yt[:, eb, :])
        else:
            self.copyback(yt[:, eb, :], pt[:])
            if eb == 1:
                self.next_store_q().dma_start(
                    self.y[ts(sc, P), :].rearrange("p (e q) -> p e q", e=2),
                    yt[:, :, :])

    def inject(self):
        if self.pe_pending:
            self.pe_pending.pop(0)()

    def load_mask(self, qb):
        if self.maskT is None:
            return None
        nc = self.nc
        mtile = self.mstg.tile([P, NM, SQ], F32, tag="mask", name=f"mask_{qb}")
        for m in range(NM):
            nc.sync.dma_start(
                mtile[:, m, :],
                self.maskT[:, :].rearrange("(m p) q -> p m q", p=P)[:, m, ds(qb * SQ, SQ)],
            )
        return mtile

    # -- main ---------------------------------------------------------------
    def run(self):
        from contextlib import ExitStack

        tc, nc = self.tc, self.nc
        # mid-run stores avoid the scalar engine (drain-critical); the tail
        # rotation uses all three DMA-capable queues
        self.store_qs = [nc.sync, nc.gpsimd]
        stack = ExitStack()
        sb = stack.enter_context(tc.tile_pool(name="sb", bufs=1))
        # PSUM budget (8 banks): sc 4 (score double-buffer), fc 2
        # (projections + injected work), av 2 (po double buffer)
        self.ps_sc = stack.enter_context(tc.tile_pool(name="ps_sc", bufs=4, space="PSUM"))
        self.ps_fc = stack.enter_context(tc.tile_pool(name="ps_fc", bufs=2, space="PSUM"))
        self.ps_av = stack.enter_context(tc.tile_pool(name="ps_av", bufs=2, space="PSUM"))
        self.attn_pool = stack.enter_context(tc.tile_pool(name="attn", bufs=2))
        self.mstg = stack.enter_context(tc.tile_pool(name="mstg", bufs=2))
        self.ystage = stack.enter_context(tc.tile_pool(name="ystage", bufs=3))

        self.xb = sb.tile([P, NQB, KD, SQ], BF16, name="xb")
        self.wq_sb = sb.tile([P, DLC, KD, P], BF16, name="wq_sb")
        self.wk_sb = sb.tile([P, DLC, KD, P], BF16, name="wk_sb")
        self.wv_sb = sb.tile([P, KD, DL], BF16, name="wv_sb")
        self.wfc_sb = sb.tile([P, DLC, D], BF16, name="wfc_sb")
        self.qT = sb.tile([P, DLC, S], BF16, name="qT")
        self.kT = sb.tile([P, DLC, S], BF16, name="kT")
        self.vN = sb.tile([P, NM, DL], BF16, name="vN")
        self.outT = sb.tile([P, DLC, S], BF16, name="outT")
        self.pe_pending = []

        # loads: ONE fifo queue in exact consumption order (ring arbitration
        # would otherwise let later transfers steal bandwidth from the x
        # stream the head is waiting on). The head emission below matches
        # this order so the PE's in-order stream never waits on a transfer
        # queued behind data it already consumed.
        def ld(dst, src):
            nc.sync.dma_start(dst, src)

        ld(self.wk_sb[:, 0, :, :], self.wk[:, 0, :, :])
        ld(self.xb[:, 0, ds(0, 4), :], self.xT[:, 0, ds(0, 4), :])
        ld(self.wk_sb[:, 1, :, :], self.wk[:, 1, :, :])
        ld(self.xb[:, 0, ds(4, 4), :], self.xT[:, 0, ds(4, 4), :])
        for kh in range(2):
            ld(self.xb[:, 1, ds(kh * 4, 4), :], self.xT[:, 1, ds(kh * 4, 4), :])
        ld(self.wq_sb[:], self.wq[:, :, :, :])
        for kh in range(2):
            ld(self.xb[:, 2, ds(kh * 4, 4), :], self.xT[:, 2, ds(kh * 4, 4), :])
        ld(self.wv_sb[:], self.wv[:, :, :])
        for kh in range(2):
            ld(self.xb[:, 3, ds(kh * 4, 4), :], self.xT[:, 3, ds(kh * 4, 4), :])
        ld(self.wfc_sb[:], self.wfc[:, :, :])

        # PE p-state warmup: dummy matmuls on a memset scratch tile run
        # during the otherwise-idle DMA wait so the ramp to 2.4GHz happens
        # before the first real projection
        scratch = sb.tile([P, SQ], BF16, name="scratch")
        nc.vector.memset(scratch[:], 0.0)
        wp = self.ps_av.tile([P, SQ], F32, tag="av", name="warmup")
        for _ in range(12):
            nc.tensor.matmul(wp[:], scratch[:, 0:P], scratch[:], start=True, stop=True)

        # serial head in arrival order: kT nb0-2, q-projection for qb0,
        # v chunks 0-11, kT nb3, v chunks 12-15
        for nb in range(3):
            for c in range(DLC):
                self.kq_group(self.wk_sb, self.kT, c, nb)
        self.kq_group(self.wq_sb, self.qT, 0, 0)
        for sc in range(11):
            self.v_group(sc)
        for c in range(DLC):
            self.kq_group(self.wk_sb, self.kT, c, 3)
        # qT0-c1 and v11-15 ride loop 0's otherwise-empty injection slots
        # (batch-2 leaves ~126ns/iteration of PE slack under the relu floor);
        # deadlines: v(m) is needed by av(m) two iterations later at the
        # earliest, qT0-c1 by loop (0,1)'s first scores pair.
        self.pe_pending = [
            lambda: self.kq_group(self.wq_sb, self.qT, 1, 0)
        ] + [lambda sc=sc: self.v_group(sc) for sc in range(11, NM)]

        # attention loops are software-pipelined ACROSS (qb, hp): each loop's
        # last two av steps and its po copyback slide into the next loop's
        # first two iterations, so the PE stream never drains at boundaries
        def po_copyback(pqb, php, ppo):
            nc.vector.tensor_copy(
                self.outT[:, php, ds(pqb * SQ, SQ // 2)], ppo[:, 0:SQ // 2])
            nc.scalar.copy(
                self.outT[:, php, ds(pqb * SQ + SQ // 2, SQ // 2)],
                ppo[:, SQ // 2:SQ])

        prev = None
        for qb in range(NQB):
            mt = self.load_mask(qb)
            for hp in range(DLC):
                at = self.attn_pool.tile(
                    [P, NM, 2, SQ], BF16, tag="attn", name=f"attn_{qb}_{hp}")
                po = self.ps_av.tile([P, SQ], F32, tag="av", name=f"av_{qb}_{hp}")
                # batch-2 geometry: two scores pairs, then two av pairs.
                # A row-quadrant <-> col-quadrant weight-tile switch costs
                # ~100ns of PE pipeline drain; batching halves the switches
                # (measured 656 -> 557 ns per iteration pure-PE), which drops
                # the PE floor below the relu-drain floor and gives injected
                # work slack to amortize into.
                for mb in range(NM // 2):
                    m0, m1 = 2 * mb, 2 * mb + 1
                    self.scores_pair(qb, hp, m0, at, mt)
                    self.scores_pair(qb, hp, m1, at, mt)
                    if mb == 0:
                        if prev is not None:
                            pqb, php, pat, ppo = prev
                            self.av(pqb, php, NM - 2, pat, ppo)
                            self.av(pqb, php, NM - 1, pat, ppo)
                            po_copyback(pqb, php, ppo)
                    else:
                        self.av(qb, hp, m0 - 2, at, po)
                        self.av(qb, hp, m1 - 2, at, po)
                        self.inject()
                prev = (qb, hp, at, po)
                if hp == 0 and qb < NQB - 1:
                    qn = qb + 1
                    self.pe_pending[:0] = [
                        (lambda c=c, qn=qn: self.kq_group(self.wq_sb, self.qT, c, qn))
                        for c in range(DLC)
                    ]
            if qb < NQB - 1:
                self.pe_pending += [
                    (lambda sc=sc, eb=eb: self.fc_group(sc, eb, tail=False))
                    for sc in range(qb * 4, qb * 4 + 4) for eb in range(2)
                ]
        # drain the final loop's av tail, then its fc
        qb3, hp3, at3, po3 = prev
        for m in (NM, NM + 1):
            self.av(qb3, hp3, m - 2, at3, po3)
        po_copyback(qb3, hp3, po3)
        self.store_qs = [nc.sync, nc.scalar, nc.gpsimd]
        self.pe_pending += [
            (lambda sc=sc, eb=eb: self.fc_group(sc, eb, tail=True))
            for sc in range(12, 16) for eb in range(2)
        ]
        while self.pe_pending:
            self.inject()

        stack.close()


# ---- host wrapper ---------------------------------------------------------

N_HEAD = 16
_nc_cache = {}


def get_nc(with_mask: bool):
    if with_mask not in _nc_cache:
        _nc_cache[with_mask] = build_nc(with_mask)
    return _nc_cache[with_mask]


def make_in_maps(x, mask, Wq, Wk, Wv, Wfc, with_mask):
    scale = np.float32(1.0 / np.sqrt(D // N_HEAD))
    bf = ml_dtypes.bfloat16
    in_maps = []
    for c in range(8):
        b, hg = divmod(c, 4)
        gs = slice(DL * hg, DL * hg + DL)
        def prearrange(wT, cdim):  # [cdim*128, F] -> [128, cdim, F]
            F = wT.shape[1]
            return np.ascontiguousarray(
                wT.reshape(cdim, P, F).transpose(1, 0, 2)
            ).astype(bf)

        def prearrange_c(wT):  # [KD*128, DLC*128] -> [128, DLC, KD, 128]
            return np.ascontiguousarray(
                wT.reshape(KD, P, DLC, P).transpose(1, 2, 0, 3)
            ).astype(bf)

        xt = x[b].T.reshape(KD, P, NQB, SQ).transpose(1, 2, 0, 3)
        m = {
            "xT": np.ascontiguousarray(xt).astype(bf),
            "wq": prearrange_c((Wq[gs, :] * scale).T),
            "wk": prearrange_c(Wk[gs, :].T),
            "wv": prearrange(Wv[gs, :].T, KD),
            "wfc": prearrange(Wfc[:, gs].T, DLC),
        }
        if with_mask:
            m["maskT"] = np.ascontiguousarray(
                np.broadcast_to(mask, (1, 1, S, S))[0, 0].T.astype(np.float32)
            )
        in_maps.append(m)
    return in_maps


def kernel(x, mask, Wq, Wk, Wv, Wfc, bfc):
    """Full-input entry: shards across 8 trn2 cores, returns the full output."""
    from concourse.bass_utils import run_bass_kernel_spmd

    x = np.asarray(x, dtype=np.float32)
    mask = np.asarray(mask, dtype=np.float32)
    Wq = np.asarray(Wq, dtype=np.float32)
    Wk = np.asarray(Wk, dtype=np.float32)
    Wv = np.asarray(Wv, dtype=np.float32)
    Wfc = np.asarray(Wfc, dtype=np.float32)
    bfc = np.asarray(bfc, dtype=np.float32)

    B = x.shape[0]
    with_mask = bool(np.any(mask))
    nc = get_nc(with_mask)
    in_maps = make_in_maps(x, mask, Wq, Wk, Wv, Wfc, with_mask)

    res = run_bass_kernel_spmd(nc, in_maps, core_ids=list(range(8)))
    parts = np.stack([np.asarray(r["y"], dtype=np.float64) for r in res.results])
    out = parts.reshape(B, 4, S, D).sum(axis=1)
    out += bfc.astype(np.float64)
    return out.astype(np.float32)

